# revision 1
# baseline (speedup 1.0000x reference)
"""Trainium2 Bass kernel for the supervised-contrastive loss (nn_KCL_69784628626020).

Strategy (8 NeuronCores, SPMD):
  - Shard anchors (rows of q, k, y) across cores: 1024 rows/core.
  - Each core computes its [1024, 8192] slab of the score matrix
    S = q_loc @ q_full^T on the tensor engine (float32r, full rate at N>=512).
  - The per-column weight w_j = 1/count(y_j) is folded into the matmul as an
    extra K=1 rank-1 update adding TAU*ln(w_j) to the scores, so that the
    scalar engine's exp(PSUM/TAU) directly produces EW_ij = exp(S_ij/TAU)*w_j.
  - Per row i:
        A_i = sum_j  EW_ij            (diag excluded)
        B_i = sum_{y_j==y_i} EW_ij    (diag excluded)
        den_i = log(A_i - B_i)
        num_i = log(kpos_i + c_i * B_i)      # c_i = count(y_i), B*c = unweighted
        loss_i = (den_i - num_i) / (c_i - 1 + K)
    A and B each come from ONE fused DVE scalar_tensor_tensor op per tile
    (compare + multiply + row-reduce).  Diagonal exclusion is data-driven
    (host-provided global row ids compared against a column iota), so the
    program is identical across cores (SPMD-safe).
  - Class counts are computed on device: row-sums of the y-equality mask give
    count(y_i) for local rows; an AllGather assembles counts for all 8192
    columns.
  - kpos_i = sum_k exp(q_i . k_ik / TAU) via fused multiply-reduce per k.
  - Final mean: per-core partial sum via a ones-matmul partition reduction;
    host adds the 8 partials (the unshard step).
"""

import numpy as np
from contextlib import ExitStack

import concourse.bass as bass
import concourse.bacc as bacc
import concourse.tile as tile
from concourse import mybir
from concourse.bass_utils import run_bass_kernel_spmd
import ml_dtypes

F32 = mybir.dt.float32
F32R = mybir.dt.float32r
F16 = mybir.dt.float16
BF16 = mybir.dt.bfloat16

TAU = 0.07
NCORES = 8


class Cfg:
    def __init__(self, N=8192, D=512, KP=8, TW=1024):
        self.N = N            # total rows (anchors)
        self.D = D            # feature dim
        self.KP = KP          # external positives per anchor
        self.TW = TW          # column tile width
        self.NL = N // NCORES     # rows per core
        self.NB = self.NL // 128  # row blocks per core
        self.NS = N // TW         # column tiles
        self.KC = D // 128        # contraction chunks
        assert self.NL % 128 == 0 and N % TW == 0 and D % 128 == 0
        assert TW % 512 == 0
        self.NCH = TW // 512      # 512-wide matmul chunks per column tile


# Engine selection knobs (tuned from traces).
STT1_ENGINES = None  # set in build_bass
STT2_ENGINES = None
KPATH_ENGINES = None


def build_bass(cfg: Cfg, stt1_eng="vector", stt2_eng="vector", k_eng="vector"):
    N, D, KP, TW = cfg.N, cfg.D, cfg.KP, cfg.TW
    NL, NB, NS, KC, NCH = cfg.NL, cfg.NB, cfg.NS, cfg.KC, cfg.NCH

    nc = bacc.Bacc("TRN2", target_bir_lowering=False, debug=False,
                   num_devices=NCORES)

    # ---- kernel I/O -------------------------------------------------------
    qT_d = nc.dram_tensor("qT", [KC, 128, N], F32R, kind="ExternalInput")
    qTl_d = nc.dram_tensor("qTl", [KC, 128, NL], F32R, kind="ExternalInput")
    kr_d = nc.dram_tensor("kr", [NB, 128, KP * D], BF16, kind="ExternalInput")
    qr_d = nc.dram_tensor("qr", [NB, 128, D], F32, kind="ExternalInput")
    ybc_d = nc.dram_tensor("ybc", [128, N], F16, kind="ExternalInput")
    yrow_d = nc.dram_tensor("yrow", [128, NB], F32, kind="ExternalInput")
    colid_d = nc.dram_tensor("colid", [128, TW], F16, kind="ExternalInput")
    rowadj_d = nc.dram_tensor("rowadj", [128, NB * NS], F32, kind="ExternalInput")
    out_d = nc.dram_tensor("out", [1, 1], F32, kind="ExternalOutput")

    eng = {"vector": nc.vector, "gpsimd": nc.gpsimd}
    stt1e = eng[stt1_eng]
    stt2e = eng[stt2_eng]
    ke = eng[k_eng]

    with tile.TileContext(nc) as tc, ExitStack() as ctx:
        const = ctx.enter_context(tc.tile_pool(name="const", bufs=1))
        rh_pool = ctx.enter_context(tc.tile_pool(name="rh", bufs=8))
        psum_pool = ctx.enter_context(tc.tile_pool(name="ps", bufs=3, space="PSUM"))
        ew_pool = ctx.enter_context(tc.tile_pool(name="ew", bufs=3))
        t1_pool = ctx.enter_context(tc.tile_pool(name="t1", bufs=3))
        t2_pool = ctx.enter_context(tc.tile_pool(name="t2", bufs=2))
        k_pool = ctx.enter_context(tc.tile_pool(name="kp", bufs=2))
        q_pool = ctx.enter_context(tc.tile_pool(name="qp", bufs=2))
        dram = ctx.enter_context(tc.tile_pool(name="dram", bufs=1, space="DRAM"))

        # ---- resident constants ------------------------------------------
        qtl = [const.tile([128, NL], F32R, tag=f"qtl{c}", name=f"qtl{c}") for c in range(KC)]
        for c in range(KC):
            nc.sync.dma_start(qtl[c][:, :], qTl_d[c, :, :])
        ybc = const.tile([128, N], F16, tag="ybc")
        nc.sync.dma_start(ybc[:, :], ybc_d[:, :])
        colid = const.tile([128, TW], F16, tag="colid")
        nc.sync.dma_start(colid[:, :], colid_d[:, :])
        yrow = const.tile([128, NB], F32, tag="yrow")
        nc.sync.dma_start(yrow[:, :], yrow_d[:, :])
        rowadj = const.tile([128, NB * NS], F32, tag="rowadj")
        nc.sync.dma_start(rowadj[:, :], rowadj_d[:, :])

        ones_k1 = const.tile([1, 128], F32R, tag="ones_k1")
        nc.vector.memset(ones_k1[:, :].bitcast(F32), 1.0)
        ones_col = const.tile([128, 1], F32, tag="ones_col")
        nc.vector.memset(ones_col[:, :], 1.0)

        # accumulator slots
        aslt = const.tile([128, NB * NS], F32, tag="aslt")
        bslt = const.tile([128, NB * NS], F32, tag="bslt")
        kss = const.tile([128, NB * KP], F32, tag="kss")
        kpos = const.tile([128, NB], F32, tag="kpos")
        cloc = const.tile([128, NB], F32, tag="cloc")
        losscol = const.tile([128, NB], F32, tag="losscol")

        # ---- phase W: class counts + lw ----------------------------------
        cnt_scr = const.tile([128, N], F16, tag="cnt_scr")
        for b in range(NB):
            nc.vector.tensor_scalar(
                cnt_scr[:, :], ybc[:, :], yrow[:, b:b + 1], None,
                op0=mybir.AluOpType.is_equal,
                op1=mybir.AluOpType.add,
                accum_out=cloc[:, b:b + 1])

        cpart = dram.tile([1, NL], F32)
        call = dram.tile([NCORES, NL], F32, addr_space="Shared")
        # cpart[0, b*128+p] = cloc[p, b]
        nc.sync.dma_start(
            cpart[:, :].rearrange("o (b p) -> p (o b)", b=NB, p=128),
            cloc[:, :])
        nc.gpsimd.collective_compute(
            "AllGather", mybir.AluOpType.bypass,
            ins=[cpart[:, :].opt()],
            outs=[call[:, :].opt()],
            replica_groups=[list(range(NCORES))],
        )
        # counts for all N columns -> SBUF [128, N/128] (global row-major)
        NF = N // 128
        csb = const.tile([128, NF], F32, tag="csb")
        nc.sync.dma_start(
            csb[:, :],
            call[:, :].rearrange("r l -> (r l)").rearrange("(p f) -> p f", p=128, f=NF))
        lnc = const.tile([128, NF], F32, tag="lnc")
        nc.scalar.activation(lnc[:, :], csb[:, :], mybir.ActivationFunctionType.Ln)
        lwsb = const.tile([128, NF], F32R, tag="lwsb")
        nc.vector.tensor_scalar_mul(lwsb[:, :], lnc[:, :], -TAU)
        lw_d = dram.tile([1, N], F32R)
        nc.sync.dma_start(
            lw_d[:, :].rearrange("o (p f) -> p (o f)", p=128, f=NF),
            lwsb[:, :])
        lwrow = const.tile([1, N], F32R, tag="lwrow")
        nc.sync.dma_start(lwrow[:, :], lw_d[:, :])

        # ---- main loop: score slab ---------------------------------------
        for s in range(NS):
            rhs = [rh_pool.tile([128, TW], F32R, tag="rh", name=f"rhs{s}_{c2}") for c2 in range(KC)]
            for c in range(KC):
                nc.sync.dma_start(rhs[c][:, :], qT_d[c, :, s * TW:(s + 1) * TW])
            for b in range(NB):
                ps = psum_pool.tile([128, TW], F32)
                for nch in range(NCH):
                    o = ps[:, nch * 512:(nch + 1) * 512]
                    for c in range(KC):
                        nc.tensor.matmul(
                            o,
                            qtl[c][:, b * 128:(b + 1) * 128],
                            rhs[c][:, nch * 512:(nch + 1) * 512],
                            start=(c == 0), stop=False)
                    nc.tensor.matmul(
                        o,
                        ones_k1[0:1, :],
                        lwrow[0:1, s * TW + nch * 512: s * TW + (nch + 1) * 512],
                        start=False, stop=True)
                ew = ew_pool.tile([128, TW], F32)
                nc.scalar.activation(ew[:, :], ps[:, :],
                                     mybir.ActivationFunctionType.Exp,
                                     scale=float(1.0 / TAU))
                # A: zero the diagonal, row-sum everything
                t1 = t1_pool.tile([128, TW], F32)
                stt1e.scalar_tensor_tensor(
                    t1[:, :], colid[:, :], rowadj[:, (b * NS + s):(b * NS + s) + 1],
                    ew[:, :],
                    op0=mybir.AluOpType.not_equal, op1=mybir.AluOpType.mult,
                    accum_out=aslt[:, (b * NS + s):(b * NS + s) + 1])
                # B: same-class row-sum (diag already zeroed in t1)
                t2 = t2_pool.tile([128, TW], F16)
                stt2e.scalar_tensor_tensor(
                    t2[:, :], ybc[:, s * TW:(s + 1) * TW], yrow[:, b:b + 1],
                    t1[:, :],
                    op0=mybir.AluOpType.is_equal, op1=mybir.AluOpType.mult,
                    accum_out=bslt[:, (b * NS + s):(b * NS + s) + 1])

        # ---- k-path: kpos = sum_k exp(q.k/TAU) ---------------------------
        for b in range(NB):
            kt = k_pool.tile([128, KP * D], BF16, tag="kt")
            nc.sync.dma_start(kt[:, :], kr_d[b, :, :])
            qt = q_pool.tile([128, D], F32, tag="qt")
            nc.sync.dma_start(qt[:, :], qr_d[b, :, :])
            for kk in range(KP):
                kscr = q_pool.tile([128, D], BF16, tag="kscr")
                ke.scalar_tensor_tensor(
                    kscr[:, :], kt[:, kk * D:(kk + 1) * D], 1.0,
                    qt[:, :],
                    op0=mybir.AluOpType.mult, op1=mybir.AluOpType.mult,
                    accum_out=kss[:, b * KP + kk: b * KP + kk + 1])
            ksse = const.tile([128, KP], F32, tag=f"ksse{b}")
            nc.scalar.activation(
                ksse[:, :],
                kss[:, b * KP:(b + 1) * KP],
                mybir.ActivationFunctionType.Exp, scale=float(1.0 / TAU),
                accum_out=kpos[:, b:b + 1])

        # ---- finalize per row block --------------------------------------
        fin = const.tile([128, 6 * NB], F32, tag="fin")
        for b in range(NB):
            acol = fin[:, 6 * b + 0: 6 * b + 1]
            bcol = fin[:, 6 * b + 1: 6 * b + 2]
            nc.vector.tensor_reduce(acol, aslt[:, b * NS:(b + 1) * NS],
                                    mybir.AxisListType.X, mybir.AluOpType.add)
            nc.vector.tensor_reduce(bcol, bslt[:, b * NS:(b + 1) * NS],
                                    mybir.AxisListType.X, mybir.AluOpType.add)
            den_in = fin[:, 6 * b + 2: 6 * b + 3]
            nc.vector.tensor_sub(den_in, acol, bcol)
            num_in = fin[:, 6 * b + 3: 6 * b + 4]
            # num_in = kpos + cloc * B
            nc.vector.scalar_tensor_tensor(
                num_in, bcol, cloc[:, b:b + 1], kpos[:, b:b + 1],
                op0=mybir.AluOpType.mult, op1=mybir.AluOpType.add)
            den_l = fin[:, 6 * b + 4: 6 * b + 5]
            nc.scalar.activation(den_l, den_in, mybir.ActivationFunctionType.Ln)
            num_l = fin[:, 6 * b + 5: 6 * b + 6]
            nc.scalar.activation(num_l, num_in, mybir.ActivationFunctionType.Ln)
        # losscol[:, b] = (den_l - num_l) / (cloc - 1 + KP)
        dinv_t = const.tile([128, NB], F32, tag="dinv")
        tmp_t = const.tile([128, NB], F32, tag="tmpd")
        nc.vector.tensor_scalar_add(tmp_t[:, :], cloc[:, :], float(KP - 1))
        nc.vector.reciprocal(dinv_t[:, :], tmp_t[:, :])
        for b in range(NB):
            den_l = fin[:, 6 * b + 4: 6 * b + 5]
            num_l = fin[:, 6 * b + 5: 6 * b + 6]
            diff = fin[:, 6 * b + 2: 6 * b + 3]  # overwrite den_in
            nc.vector.tensor_sub(diff, den_l, num_l)
            nc.vector.tensor_mul(losscol[:, b:b + 1], diff, dinv_t[:, b:b + 1])

        # ---- reduce to a single partial ----------------------------------
        lsum = const.tile([128, 1], F32, tag="lsum")
        nc.vector.tensor_reduce(lsum[:, :], losscol[:, :],
                                mybir.AxisListType.X, mybir.AluOpType.add)
        psf = psum_pool.tile([128, 512], F32, bufs=1)
        nc.tensor.matmul(psf[0:1, 0:1], lsum[:, :],
                         ones_col[:, :], start=True, stop=True)
        outsb = const.tile([1, 1], F32, tag="outsb")
        nc.scalar.copy(outsb[0:1, 0:1], psf[0:1, 0:1])
        nc.sync.dma_start(out_d[:, :], outsb[0:1, 0:1])

    nc.compile()
    return nc


# ---------------------------------------------------------------------------
# host-side marshalling
# ---------------------------------------------------------------------------

def make_inputs(q, k, y, cfg: Cfg):
    """Build the per-core input maps (pure layout/replication marshalling)."""
    N, D, KP, TW = cfg.N, cfg.D, cfg.KP, cfg.TW
    NL, NB, NS, KC = cfg.NL, cfg.NB, cfg.NS, cfg.KC
    q = np.asarray(q, dtype=np.float32)
    k = np.asarray(k, dtype=np.float32)
    y = np.asarray(y)

    qT = np.ascontiguousarray(q.T).reshape(KC, 128, N)
    ybc = np.broadcast_to(y.astype(np.float16)[None, :], (128, N)).copy()
    colid = np.broadcast_to(np.arange(TW, dtype=np.float16)[None, :], (128, TW)).copy()

    in_maps = []
    for r in range(NCORES):
        rows = slice(r * NL, (r + 1) * NL)
        qTl = np.ascontiguousarray(q[rows].T).reshape(KC, 128, NL)
        kr = np.ascontiguousarray(k[rows].reshape(NB, 128, KP * D)).astype(ml_dtypes.bfloat16)
        qr = np.ascontiguousarray(q[rows].reshape(NB, 128, D))
        yrow = np.ascontiguousarray(y[rows].astype(np.float32).reshape(NB, 128).T)
        # rowadj[p, b*NS+s] = global_row - s*TW
        p = np.arange(128, dtype=np.float32)
        badx = np.arange(NB, dtype=np.float32)
        sadx = np.arange(NS, dtype=np.float32)
        grow = r * NL + badx[:, None, None] * 128 + p[None, :, None]  # [NB,128,1]
        rowadj = (grow - sadx[None, None, :] * TW)                   # [NB,128,NS]
        rowadj = np.ascontiguousarray(rowadj.transpose(1, 0, 2).reshape(128, NB * NS),
                                      dtype=np.float32)
        in_maps.append({
            "qT": qT, "qTl": qTl, "kr": kr, "qr": qr,
            "ybc": ybc, "yrow": yrow, "colid": colid, "rowadj": rowadj,
        })
    return in_maps


_CACHE = {}


def _get_nc(cfg_key):
    if cfg_key not in _CACHE:
        cfg = Cfg()
        _CACHE[cfg_key] = (cfg, build_bass(cfg))
    return _CACHE[cfg_key]


def kernel(q, k, y, trace=False):
    cfg, nc = _get_nc("full")
    in_maps = make_inputs(q, k, y, cfg)
    res = run_bass_kernel_spmd(nc, in_maps, core_ids=list(range(NCORES)),
                               trace=trace)
    total = np.sum([res.results[r]["out"][0, 0] for r in range(NCORES)],
                   dtype=np.float64)
    out = np.asarray(total / cfg.N, dtype=np.float32)
    if trace:
        kernel.last_results = res
    return out



# revision 9
# speedup vs baseline: 1.5955x; 1.5955x over previous
"""Trainium2 Bass kernel for the supervised-contrastive loss (nn_KCL_69784628626020).

Strategy (8 NeuronCores, SPMD):
  - Shard anchors (rows of q, k, y) across cores: 1024 rows/core.
  - Each core computes its [1024, 8192] slab of S = q_loc @ q_full^T with
    bf16 matmuls (full PE rate; fp32r measured 2x slower on HW).
  - Diagonal exclusion is done IN PSUM: one extra bf16 matmul per tile adds
    -30 to the diagonal entry (lhsT = zsel, a core-specific -30*I block;
    rhs = a shifted-identity window W).  exp((S-30)/tau) == 0 exactly, so
    all downstream row-sums are diagonal-free with no masking pass.
  - E = exp(S/tau) in bf16 (scalar engine), buffered deep in SBUF.
  - Per row i:
        AW_i = sum_j E_ij * w_j          (w_j = 1/count(y_j); TTR on DVE)
        BU_i = sum_{y_j==y_i} E_ij       (STT compare+mult on DVE/Pool)
        den_i = log(AW_i - w_i*BU_i)
        num_i = log(kpos_i + BU_i)
        loss_i = (den_i - num_i) / (count_i - 1 + K)
  - Class counts: per-local-row compare+reduce chunks split across DVE and
    Pool, AllGather assembles all 8192 counts; w broadcast to [128, N] via
    a K=1 PE matmul.  Nothing in the matmul/exp/BU pipeline waits on this;
    only the AW accumulation does, and it catches up (E tiles buffer).
  - kpos_i = sum_k exp(q_i . k_ik / tau) via fused multiply-reduce per k.
  - Final mean: per-core partial via ones-matmul partition reduction; host
    adds the 8 partials (the unshard step).
"""

import numpy as np
from contextlib import ExitStack

import concourse.bass as bass
import concourse.bacc as bacc
import concourse.tile as tile
from concourse import mybir
from concourse.bass_utils import run_bass_kernel_spmd
import ml_dtypes

F32 = mybir.dt.float32
F32R = mybir.dt.float32r
F16 = mybir.dt.float16
BF16 = mybir.dt.bfloat16

TAU = 0.07
NCORES = 8
DIAG_C = 30.0


class Cfg:
    def __init__(self, N=8192, D=512, KP=8, TW=1024, ncores=NCORES):
        self.N = N            # total rows (anchors)
        self.D = D            # feature dim
        self.KP = KP          # external positives per anchor
        self.TW = TW          # column tile width
        self.ncores = ncores
        self.NL = N // ncores     # rows per core
        self.NB = self.NL // 128  # row blocks per core
        self.NS = N // TW         # column tiles
        self.KC = D // 128        # contraction chunks
        assert self.NL % 128 == 0 and N % TW == 0 and D % 128 == 0
        assert TW % 512 == 0
        # diag-kill geometry requires each core's diagonal blocks to live in
        # a single column tile at matching offsets
        assert self.NL == TW
        self.NCH = TW // 512      # 512-wide matmul chunks per column tile
        self.CQ = 4               # count pass: quarters per row-block
        assert TW % 2 == 0 and N % (self.CQ) == 0


# ---- engine assignment knobs (tuned from traces) --------------------------
# count chunk i (of NB*CQ) -> engine;  k-path (b,kk) -> engine;
# BU (b,s) -> engine; AW (b,s) -> engine.
def cnt_eng(i, n):
    return "vector"


def k_eng(b, kk):
    return "vector"


def bu_eng(b, s):
    return "vector"


def aw_eng(b, s):
    return "vector"


def build_bass(cfg: Cfg, e_bufs=32):
    N, D, KP, TW = cfg.N, cfg.D, cfg.KP, cfg.TW
    NL, NB, NS, KC, NCH, CQ = cfg.NL, cfg.NB, cfg.NS, cfg.KC, cfg.NCH, cfg.CQ

    nc = bacc.Bacc("TRN2", target_bir_lowering=False, debug=False,
                   num_devices=cfg.ncores)

    # ---- kernel I/O -------------------------------------------------------
    qT_d = nc.dram_tensor("qT", [KC, 128, N], BF16, kind="ExternalInput")
    qTl_d = nc.dram_tensor("qTl", [KC, 128, NL], BF16, kind="ExternalInput")
    kr_d = nc.dram_tensor("kr", [NB, 128, KP * D], BF16, kind="ExternalInput")
    qr_d = nc.dram_tensor("qr", [NB, 128, D], BF16, kind="ExternalInput")
    ybc_d = nc.dram_tensor("ybc", [128, N], F16, kind="ExternalInput")
    yrow_d = nc.dram_tensor("yrow", [128, NB], F32, kind="ExternalInput")
    wdg_d = nc.dram_tensor("wdg", [128, TW + (NB - 1) * 128], BF16,
                           kind="ExternalInput")
    zsel_d = nc.dram_tensor("zsel", [128, NS * 128], BF16, kind="ExternalInput")
    out_d = nc.dram_tensor("out", [1, 1], F32, kind="ExternalOutput")

    eng = {"vector": nc.vector, "gpsimd": nc.gpsimd}

    with tile.TileContext(nc) as tc, ExitStack() as ctx:
        const = ctx.enter_context(tc.tile_pool(name="const", bufs=1))
        rh_pool = ctx.enter_context(tc.tile_pool(name="rh", bufs=2))
        psum_pool = ctx.enter_context(tc.tile_pool(name="ps", bufs=3, space="PSUM"))
        psb_pool = ctx.enter_context(tc.tile_pool(name="psb", bufs=1, space="PSUM"))
        ew_pool = ctx.enter_context(tc.tile_pool(name="ew", bufs=e_bufs))
        awsc_pool = ctx.enter_context(tc.tile_pool(name="awsc", bufs=2))
        busc_pool = ctx.enter_context(tc.tile_pool(name="busc", bufs=2))
        cnt_pool = ctx.enter_context(tc.tile_pool(name="cnt", bufs=2))
        k_pool = ctx.enter_context(tc.tile_pool(name="kp", bufs=2))
        q_pool = ctx.enter_context(tc.tile_pool(name="qp", bufs=2))
        ks_pool = ctx.enter_context(tc.tile_pool(name="ks", bufs=2))
        dram = ctx.enter_context(tc.tile_pool(name="dram", bufs=1, space="DRAM"))

        # ---- resident constants ------------------------------------------
        qtl = [const.tile([128, NL], BF16, tag=f"qtl{c}", name=f"qtl{c}")
               for c in range(KC)]
        for c in range(KC):
            nc.sync.dma_start(qtl[c][:, :], qTl_d[c, :, :])
        ybc = const.tile([128, N], F16, tag="ybc")
        nc.sync.dma_start(ybc[:, :], ybc_d[:, :])
        yrow = const.tile([128, NB], F32, tag="yrow")
        nc.sync.dma_start(yrow[:, :], yrow_d[:, :])
        wdg = const.tile([128, TW + (NB - 1) * 128], BF16, tag="wdg")
        nc.sync.dma_start(wdg[:, :], wdg_d[:, :])
        zsel = const.tile([128, NS * 128], BF16, tag="zsel")
        nc.sync.dma_start(zsel[:, :], zsel_d[:, :])

        ones_k1 = const.tile([1, 128], BF16, tag="ones_k1")
        nc.vector.memset(ones_k1[:, :], 1.0)
        ones_col = const.tile([128, 1], F32, tag="ones_col")
        nc.vector.memset(ones_col[:, :], 1.0)

        # accumulator slots
        awslt = const.tile([128, NB * NS], F32, tag="awslt")
        buslt = const.tile([128, NB * NS], F32, tag="buslt")
        cslt = const.tile([128, NB * CQ], F32, tag="cslt")
        kss = const.tile([128, NB * KP], F32, tag="kss")
        kpos = const.tile([128, NB], F32, tag="kpos")
        cloc = const.tile([128, NB], F32, tag="cloc")
        losscol = const.tile([128, NB], F32, tag="losscol")
        wbc = const.tile([128, N], BF16, tag="wbc")

        # ---- phase W: class counts (split DVE/Pool), AllGather, w --------
        CW = N // CQ
        ncnt = NB * CQ
        for i in range(ncnt):
            b, qq = divmod(i, CQ)
            e = eng[cnt_eng(i, ncnt)]
            scr = cnt_pool.tile([128, CW], BF16, tag=f"cscr_{cnt_eng(i, ncnt)}")
            e.tensor_scalar(
                scr[:, :], ybc[:, qq * CW:(qq + 1) * CW], yrow[:, b:b + 1], None,
                op0=mybir.AluOpType.is_equal,
                op1=mybir.AluOpType.add,
                accum_out=cslt[:, i:i + 1])
        for b in range(NB):
            nc.vector.tensor_reduce(cloc[:, b:b + 1], cslt[:, b * CQ:(b + 1) * CQ],
                                    mybir.AxisListType.X, mybir.AluOpType.add)

        cpart = dram.tile([1, NL], F32)
        call = dram.tile([cfg.ncores, NL], F32, addr_space="Shared")
        # cpart[0, b*128+p] = cloc[p, b]
        nc.sync.dma_start(
            cpart[:, :].rearrange("o (b p) -> p (o b)", b=NB, p=128),
            cloc[:, :])
        nc.gpsimd.collective_compute(
            "AllGather", mybir.AluOpType.bypass,
            ins=[cpart[:, :].opt()],
            outs=[call[:, :].opt()],
            replica_groups=[list(range(cfg.ncores))],
        )
        # counts for all N columns -> SBUF [128, N/128] (global row-major)
        NF = N // 128
        csb = const.tile([128, NF], F32, tag="csb")
        nc.sync.dma_start(
            csb[:, :],
            call[:, :].rearrange("r l -> (r l)").rearrange("(p f) -> p f", p=128, f=NF))
        wsb = const.tile([128, NF], F32, tag="wsb")
        nc.vector.reciprocal(wsb[:, :], csb[:, :])
        wsb16 = const.tile([128, NF], BF16, tag="wsb16")
        nc.vector.tensor_copy(wsb16[:, :], wsb[:, :])
        w_d = dram.tile([1, N], BF16)
        nc.sync.dma_start(
            w_d[:, :].rearrange("o (p f) -> p (o f)", p=128, f=NF),
            wsb16[:, :])
        # broadcast w to all partitions (gpsimd ISA op; DVE/Act stay free)
        wrow = const.tile([1, N], BF16, tag="wrow")
        nc.sync.dma_start(wrow[:, :], w_d[:, :])
        nc.gpsimd.partition_broadcast(wbc[:, :], wrow[:, :])

        # local per-row factors
        nwloc = const.tile([128, NB], F32, tag="nwloc")   # -1/count
        dinv = const.tile([128, NB], F32, tag="dinv")     # 1/(count-1+KP)
        tmp_t = const.tile([128, NB], F32, tag="tmpd")
        nc.vector.reciprocal(tmp_t[:, :], cloc[:, :])
        nc.vector.tensor_scalar_mul(nwloc[:, :], tmp_t[:, :], -1.0)
        nc.vector.tensor_scalar_add(tmp_t[:, :], cloc[:, :], float(KP - 1))
        nc.vector.reciprocal(dinv[:, :], tmp_t[:, :])

        # ---- k-path: kpos = sum_k exp(q.k/TAU) ---------------------------
        for b in range(NB):
            kt = k_pool.tile([128, KP * D], BF16, tag="kt")
            nc.sync.dma_start(kt[:, :], kr_d[b, :, :])
            qt = q_pool.tile([128, D], BF16, tag="qt")
            nc.sync.dma_start(qt[:, :], qr_d[b, :, :])
            for kk in range(KP):
                e = eng[k_eng(b, kk)]
                kscr = ks_pool.tile([128, D], BF16, tag=f"kscr_{k_eng(b, kk)}")
                e.scalar_tensor_tensor(
                    kscr[:, :], kt[:, kk * D:(kk + 1) * D], 1.0,
                    qt[:, :],
                    op0=mybir.AluOpType.mult, op1=mybir.AluOpType.mult,
                    accum_out=kss[:, b * KP + kk: b * KP + kk + 1])
            ksse = const.tile([128, KP], F32, tag=f"ksse{b}")
            nc.scalar.activation(
                ksse[:, :],
                kss[:, b * KP:(b + 1) * KP],
                mybir.ActivationFunctionType.Exp, scale=float(1.0 / TAU),
                accum_out=kpos[:, b:b + 1])

        # ---- main loop: score slab ---------------------------------------
        for s in range(NS):
            rhs = rh_pool.tile([128, KC * TW], BF16, tag="rh", name=f"rhs{s}")
            for c in range(KC):
                nc.sync.dma_start(rhs[:, c * TW:(c + 1) * TW],
                                  qT_d[c, :, s * TW:(s + 1) * TW])
            for b in range(NB):
                ps = psum_pool.tile([128, TW], F32)
                for c in range(KC):
                    for nch in range(NCH):
                        nc.tensor.matmul(
                            ps[:, nch * 512:(nch + 1) * 512],
                            qtl[c][:, b * 128:(b + 1) * 128],
                            rhs[:, c * TW + nch * 512:c * TW + (nch + 1) * 512],
                            start=(c == 0), stop=False)
                # diagonal kill: adds -DIAG_C at column (b*128+p) iff s==r
                for nch in range(NCH):
                    nc.tensor.matmul(
                        ps[:, nch * 512:(nch + 1) * 512],
                        zsel[:, s * 128:(s + 1) * 128],
                        wdg[:, (NB - 1 - b) * 128 + nch * 512:
                            (NB - 1 - b) * 128 + (nch + 1) * 512],
                        start=False, stop=True)
                ew = ew_pool.tile([128, TW], BF16)
                nc.scalar.activation(ew[:, :], ps[:, :],
                                     mybir.ActivationFunctionType.Exp,
                                     scale=float(1.0 / TAU))
                # BU: same-class row-sum (diag already zero)
                ebu = eng[bu_eng(b, s)]
                buscr = busc_pool.tile([128, TW], BF16, tag=f"buscr_{bu_eng(b, s)}")
                ebu.scalar_tensor_tensor(
                    buscr[:, :], ybc[:, s * TW:(s + 1) * TW], yrow[:, b:b + 1],
                    ew[:, :],
                    op0=mybir.AluOpType.is_equal, op1=mybir.AluOpType.mult,
                    accum_out=buslt[:, (b * NS + s):(b * NS + s) + 1])
                # AW: weighted row-sum
                ea = aw_eng(b, s)
                awscr = awsc_pool.tile([128, TW], BF16, tag=f"awscr_{ea}")
                eng[ea].scalar_tensor_tensor(
                    awscr[:, :], ew[:, :], 1.0, wbc[:, s * TW:(s + 1) * TW],
                    op0=mybir.AluOpType.mult, op1=mybir.AluOpType.mult,
                    accum_out=awslt[:, (b * NS + s):(b * NS + s) + 1])

        # ---- finalize per row block --------------------------------------
        fin = const.tile([128, 6 * NB], F32, tag="fin")
        for b in range(NB):
            awcol = fin[:, 6 * b + 0: 6 * b + 1]
            bucol = fin[:, 6 * b + 1: 6 * b + 2]
            nc.vector.tensor_reduce(awcol, awslt[:, b * NS:(b + 1) * NS],
                                    mybir.AxisListType.X, mybir.AluOpType.add)
            nc.vector.tensor_reduce(bucol, buslt[:, b * NS:(b + 1) * NS],
                                    mybir.AxisListType.X, mybir.AluOpType.add)
            den_in = fin[:, 6 * b + 2: 6 * b + 3]
            # den_in = aw + (-1/c) * bu
            nc.vector.scalar_tensor_tensor(
                den_in, bucol, nwloc[:, b:b + 1], awcol,
                op0=mybir.AluOpType.mult, op1=mybir.AluOpType.add)
            num_in = fin[:, 6 * b + 3: 6 * b + 4]
            nc.vector.tensor_add(num_in, bucol, kpos[:, b:b + 1])
            den_l = fin[:, 6 * b + 4: 6 * b + 5]
            nc.scalar.activation(den_l, den_in, mybir.ActivationFunctionType.Ln)
            num_l = fin[:, 6 * b + 5: 6 * b + 6]
            nc.scalar.activation(num_l, num_in, mybir.ActivationFunctionType.Ln)
        for b in range(NB):
            den_l = fin[:, 6 * b + 4: 6 * b + 5]
            num_l = fin[:, 6 * b + 5: 6 * b + 6]
            diff = fin[:, 6 * b + 2: 6 * b + 3]  # overwrite den_in
            nc.vector.tensor_sub(diff, den_l, num_l)
            nc.vector.tensor_mul(losscol[:, b:b + 1], diff, dinv[:, b:b + 1])

        # ---- reduce to a single partial ----------------------------------
        lsum = const.tile([128, 1], F32, tag="lsum")
        nc.vector.tensor_reduce(lsum[:, :], losscol[:, :],
                                mybir.AxisListType.X, mybir.AluOpType.add)
        psf = psb_pool.tile([128, 512], F32)
        nc.tensor.matmul(psf[0:1, 0:1], lsum[:, :],
                         ones_col[:, :], start=True, stop=True)
        outsb = const.tile([1, 1], F32, tag="outsb")
        nc.scalar.copy(outsb[0:1, 0:1], psf[0:1, 0:1])
        nc.sync.dma_start(out_d[:, :], outsb[0:1, 0:1])

    nc.compile()
    return nc


# ---------------------------------------------------------------------------
# host-side marshalling
# ---------------------------------------------------------------------------

def make_inputs(q, k, y, cfg: Cfg):
    """Build the per-core input maps (pure layout/replication marshalling)."""
    N, D, KP, TW = cfg.N, cfg.D, cfg.KP, cfg.TW
    NL, NB, NS, KC = cfg.NL, cfg.NB, cfg.NS, cfg.KC
    q = np.asarray(q, dtype=np.float32)
    k = np.asarray(k, dtype=np.float32)
    y = np.asarray(y)

    qbf = q.astype(ml_dtypes.bfloat16)
    qT = np.ascontiguousarray(qbf.T).reshape(KC, 128, N)
    ybc = np.broadcast_to(y.astype(np.float16)[None, :], (128, N)).copy()

    # wdg[q, t] = 1.0 iff t == (NB-1)*128 + q (shifted identity window)
    WDGW = TW + (NB - 1) * 128
    wdg = np.zeros((128, WDGW), dtype=ml_dtypes.bfloat16)
    for qq in range(128):
        wdg[qq, (NB - 1) * 128 + qq] = 1.0

    in_maps = []
    for r in range(cfg.ncores):
        rows = slice(r * NL, (r + 1) * NL)
        qTl = np.ascontiguousarray(qbf[rows].T).reshape(KC, 128, NL)
        kr = np.ascontiguousarray(k[rows].reshape(NB, 128, KP * D)).astype(ml_dtypes.bfloat16)
        qr = np.ascontiguousarray(qbf[rows].reshape(NB, 128, D))
        yrow = np.ascontiguousarray(y[rows].astype(np.float32).reshape(NB, 128).T)
        # zsel[p, s*128+pp] = -DIAG_C * (p==pp) * (s==r_tile) where r_tile is
        # the column tile holding this core's diagonal block(s).
        zsel = np.zeros((128, NS * 128), dtype=ml_dtypes.bfloat16)
        for b in range(NB):
            g0 = r * NL + b * 128          # global row of partition 0, block b
            s = g0 // TW                    # column tile containing the diag
            # within-tile column of the diag for partition p is g0 - s*TW + p;
            # the wdg window for block b provides [col == b*128 + p], and
            # b*128 == g0 - r*NL; this matches only when s*TW == r*NL... for
            # TW=1024=NL it does.  zsel block s set to -C*I.
            np.fill_diagonal(zsel[:, s * 128:(s + 1) * 128], -DIAG_C)
        in_maps.append({
            "qT": qT, "qTl": qTl, "kr": kr, "qr": qr,
            "ybc": ybc, "yrow": yrow, "wdg": wdg, "zsel": zsel,
        })
    return in_maps


_CACHE = {}


def _get_nc(cfg_key):
    if cfg_key not in _CACHE:
        cfg = Cfg()
        _CACHE[cfg_key] = (cfg, build_bass(cfg))
    return _CACHE[cfg_key]


def kernel(q, k, y, trace=False):
    cfg, nc = _get_nc("full")
    in_maps = make_inputs(q, k, y, cfg)
    res = run_bass_kernel_spmd(nc, in_maps, core_ids=list(range(NCORES)),
                               trace=trace)
    total = np.sum([res.results[r]["out"][0, 0] for r in range(NCORES)],
                   dtype=np.float64)
    out = np.asarray(total / cfg.N, dtype=np.float32)
    if trace:
        kernel.last_results = res
    return out
